# revision 1
# baseline (speedup 1.0000x reference)
"""DeepSet encoder (phi MLP -> sum/max pool -> rho MLP) as a Trainium2 Bass kernel.

Sharding: data-parallel over the batch dim. 64 samples -> 8 cores x 8 samples.
Weights are replicated on every core; no cross-core communication.

On-chip layout is feature-major ("transposed"): activations live as
[feature_partition, set_free] tiles so that
  - matmul contraction (over features) is on the partition dim,
  - the bias is a per-partition scalar (free on ScalarE's activation op),
  - sum/max pooling over the set dim is a free-axis reduction
    (sum comes for free via activation's accum_out).
The host pre-transposes x to [B, D_IN, N] and casts inputs to bf16.

Self-contained: only relies on the system-installed concourse/bass stack.
"""

import sys

import numpy as np

for _p in ("/opt/trn_rl_repo",):
    if _p not in sys.path:
        sys.path.insert(0, _p)

import ml_dtypes  # noqa: E402

import concourse.bass as bass  # noqa: E402,F401
import concourse.mybir as mybir  # noqa: E402
import concourse.tile as tile  # noqa: E402
from concourse import bacc  # noqa: E402
from concourse.bass_utils import run_bass_kernel_spmd  # noqa: E402

# 16-bit compute dtype: fp16 runs the PE at the same 1 cycle/row as bf16 but
# carries 10 mantissa bits instead of 8. All intermediates here are O(100) max,
# far inside fp16 range, so fp16 is a free 4x accuracy win over bf16.
BF16 = mybir.dt.float16
FP32 = mybir.dt.float32
NP_BF16 = np.float16
# phi1 runs in fp8e4m3 with DoubleRow: 2 fp8 weights per PE cell contract 256
# rows per pass, halving phi1's matmul count. x ~ N(0,1) and W1 ~ U(+-0.044)
# sit far inside TRN e4m3's +-240 range; measured end-to-end error with
# fp8-phi1 + fp16-rest is ~0.003 (same as an all-bf16 kernel).
FP8 = mybir.dt.float8e4
NP_FP8 = ml_dtypes.float8_e4m3
DOUBLE_ROW = mybir.MatmulPerfMode.DoubleRow

B, N, D_IN, D_H = 64, 512, 512, 1024
N_CORES = 8
BL = B // N_CORES  # samples per core
P = 128
K1 = D_IN // P  # phi1 contraction tiles (4)
K2 = D_H // P  # phi2/rho2 contraction tiles & D_H output tiles (8)
KR1 = 2 * D_H // P  # rho1 contraction tiles (16)
KK1 = D_IN // 256  # phi1 DoubleRow chunks (2)

RELU = mybir.ActivationFunctionType.Relu
AX_X = mybir.AxisListType.X
OP_MAX = mybir.AluOpType.max


def build_program() -> bacc.Bacc:
    nc = bacc.Bacc("TRN2", target_bir_lowering=False, debug=False, num_devices=N_CORES)

    # all staged host-side into the exact SBUF tile layouts so every DMA is
    # contiguous per partition (large descriptor runs):
    #   xt[b, p, kk, j, n] = x[b, n, kk*256 + j*128 + p]   (fp8, DoubleRow pairs)
    #   w1[p, kk, j, h] = W1[kk*256 + j*128 + p, h]        (fp8)
    #   w*[p, ko, h] = W[ko*128+p, h]                      (fp16)
    xt_d = nc.dram_tensor("xt", [BL, P, KK1, 2, N], FP8, kind="ExternalInput").ap()
    w1_d = nc.dram_tensor("w1", [P, KK1, 2, D_H], FP8, kind="ExternalInput").ap()
    w2_d = nc.dram_tensor("w2", [P, K2, D_H], BF16, kind="ExternalInput").ap()
    wr1_d = nc.dram_tensor("wr1", [P, KR1, D_H], BF16, kind="ExternalInput").ap()
    wr2_d = nc.dram_tensor("wr2", [P, K2, D_H], BF16, kind="ExternalInput").ap()
    # biases staged on host as [P, n_tiles]: b_sb[p, m] = b[m*128 + p]
    b1_d = nc.dram_tensor("b1", [P, K2], FP32, kind="ExternalInput").ap()
    b2_d = nc.dram_tensor("b2", [P, K2], FP32, kind="ExternalInput").ap()
    br1_d = nc.dram_tensor("br1", [P, K2], FP32, kind="ExternalInput").ap()
    br2_d = nc.dram_tensor("br2", [P, K2], FP32, kind="ExternalInput").ap()
    # out[p, m, s] = r2[m*128 + p, s]  (feature-major, host transposes back)
    out_d = nc.dram_tensor("out", [P, K2, BL], FP32, kind="ExternalOutput").ap()

    with tile.TileContext(nc) as tc:
        with (
            tc.tile_pool(name="const", bufs=1) as cpool,
            tc.tile_pool(name="xt", bufs=3) as xtpool,
            tc.tile_pool(name="h1", bufs=2) as h1pool,
            tc.tile_pool(name="h2", bufs=4) as h2pool,
            tc.tile_pool(name="ps", bufs=8, space="PSUM") as pspool,
        ):
            # --- PE warm-up ---
            # The PE clock sits at 1.2GHz (HAM-throttled) until ~3.4us of
            # sustained activity. Burn that window on dummy matmuls over a
            # zeroed scratch tile while the startup DMAs are in flight, so
            # the real matmuls run at 2.4GHz from the first one.
            warm_sb = cpool.tile([P, N], BF16)
            nc.gpsimd.memset(warm_sb[:], 0.0)
            for i in range(12):
                wps = pspool.tile([P, N], FP32, tag="ps", name=f"warm{i}")
                nc.tensor.matmul(wps[:], warm_sb[:, 0:P], warm_sb[:], start=True, stop=True)

            # --- persistent SBUF state ---
            # startup-critical DMAs first: the sync sequencer issues one
            # DIRECT2D per ~0.6us, so issue order = time order. Interleave
            # per-k parts of xt[0] and w1 so the first matmuls can begin
            # after ~400KB instead of ~4MB; everything else queues behind.
            w1_sb = cpool.tile([P, KK1, 2, D_H], FP8)
            xt0_sb = xtpool.tile([P, KK1, 2, N], FP8, tag="xt", name="xt0")
            xt1_sb = xtpool.tile([P, KK1, 2, N], FP8, tag="xt", name="xt1")
            for kk in range(KK1):
                nc.sync.dma_start(xt0_sb[:, kk], xt_d[0, :, kk])
                nc.sync.dma_start(w1_sb[:, kk], w1_d[:, kk])
            w2_sb = cpool.tile([P, K2, D_H], BF16)
            nc.sync.dma_start(w2_sb[:, : K2 // 2], w2_d[:, : K2 // 2])
            b1_sb = cpool.tile([P, K2], FP32)
            nc.sync.dma_start(b1_sb[:], b1_d)
            nc.sync.dma_start(xt1_sb[:], xt_d[1])
            nc.sync.dma_start(w2_sb[:, K2 // 2 :], w2_d[:, K2 // 2 :])
            b2_sb = cpool.tile([P, K2], FP32)
            nc.sync.dma_start(b2_sb[:], b2_d)

            pooled = cpool.tile([P, KR1, BL], FP32)  # [0:K2]=sum, [K2:]=max
            pooled_bf = cpool.tile([P, KR1, BL], BF16)
            r1_sb = cpool.tile([P, K2, BL], BF16)
            out_sb = cpool.tile([P, K2, BL], FP32)

            def phi1_mm(ps, m, kk, xt_sb, start, stop):
                # fp8 DoubleRow: lhsT [128, 2, 128], rhs [128, 2, 512];
                # contracts 256 input-feature rows per pass.
                nc.tensor.matmul(
                    ps[:],
                    w1_sb[:, kk, :, m * P : (m + 1) * P],
                    xt_sb[:, kk],
                    perf_mode=DOUBLE_ROW,
                    start=start,
                    stop=stop,
                )

            def phi1(b):
                if b == 0:
                    xt_sb = xt0_sb
                elif b == 1:
                    xt_sb = xt1_sb
                else:
                    xt_sb = xtpool.tile([P, KK1, 2, N], FP8, tag="xt", name=f"xt{b}")
                    nc.sync.dma_start(xt_sb[:], xt_d[b])
                h1_sb = h1pool.tile([P, K2, N], BF16, tag="h1", name=f"h1_{b}")
                if b == 0:
                    # two half-k accumulations across all m so the first 8
                    # matmuls only need the first halves of the xt0/w1 DMAs.
                    ps1 = []
                    for m in range(K2):
                        ps = pspool.tile([P, N], FP32, tag="ps", name=f"ps1_0_{m}")
                        ps1.append(ps)
                        phi1_mm(ps, m, 0, xt_sb, start=True, stop=False)
                    for m in range(K2):
                        ps = ps1[m]
                        phi1_mm(ps, m, 1, xt_sb, start=False, stop=True)
                        nc.scalar.activation(
                            h1_sb[:, m, :], ps[:], RELU,
                            bias=b1_sb[:, m : m + 1], scale=1.0,
                        )
                    return h1_sb
                for m in range(K2):
                    ps = pspool.tile([P, N], FP32, tag="ps", name=f"ps1_{b}_{m}")
                    for kk in range(KK1):
                        phi1_mm(ps, m, kk, xt_sb, start=(kk == 0), stop=(kk == KK1 - 1))
                    nc.scalar.activation(
                        h1_sb[:, m, :], ps[:], RELU, bias=b1_sb[:, m : m + 1], scale=1.0
                    )
                return h1_sb

            def phi2(b, h1_sb):
                for m in range(K2):
                    ps = pspool.tile([P, N], FP32, tag="ps", name=f"ps2_{b}_{m}")
                    for k in range(K2):
                        nc.tensor.matmul(
                            ps[:],
                            w2_sb[:, k, m * P : (m + 1) * P],
                            h1_sb[:, k, :],
                            start=(k == 0),
                            stop=(k == K2 - 1),
                        )
                    h2_sb = h2pool.tile([P, N], BF16, tag="h2", name=f"h2_{b}_{m}")
                    # relu(psum + bias) -> h2 tile; sum over set dim lands in
                    # pooled[:, m, b] via the activation accumulator.
                    nc.scalar.activation(
                        h2_sb[:],
                        ps[:],
                        RELU,
                        bias=b2_sb[:, m : m + 1],
                        scale=1.0,
                        accum_out=pooled[:, m, b : b + 1],
                    )
                    if b == BL - 1:
                        # last sample: the sum feature tile is complete as soon
                        # as the ACT accumulator lands -> cast it before the
                        # max reduce so rho1's sum-half matmuls can start.
                        nc.vector.tensor_copy(pooled_bf[:, m, :], pooled[:, m, :])
                    nc.vector.tensor_reduce(
                        pooled[:, K2 + m, b : b + 1], h2_sb[:], axis=AX_X, op=OP_MAX
                    )
                    if b == BL - 1:
                        nc.vector.tensor_copy(
                            pooled_bf[:, K2 + m, :], pooled[:, K2 + m, :]
                        )

            # software pipeline: phi1(b+1) is emitted before phi2(b) so the PE
            # never waits on the phi1->phi2 evacuation inside one sample.
            prev_h1 = None
            for b in range(BL):
                h1_sb = phi1(b)
                if prev_h1 is not None:
                    phi2(b - 1, prev_h1)
                prev_h1 = h1_sb
            phi2(BL - 1, prev_h1)

            # --- rho MLP over the 8 pooled vectors (feature-major, N=8) ---
            wr1_sb = cpool.tile([P, KR1, D_H], BF16)
            nc.sync.dma_start(wr1_sb[:], wr1_d)
            wr2_sb = cpool.tile([P, K2, D_H], BF16)
            nc.sync.dma_start(wr2_sb[:], wr2_d)
            br1_sb = cpool.tile([P, K2], FP32)
            nc.sync.dma_start(br1_sb[:], br1_d)
            br2_sb = cpool.tile([P, K2], FP32)
            nc.sync.dma_start(br2_sb[:], br2_d)

            # rho1 in two half-accumulations over all 8 m-tiles: the sum-half
            # (k=0..7) only needs the ACT accumulators, so its matmuls chase
            # the phi2 epilogue while the max reduces are still draining.
            ps_r1 = []
            for m in range(K2):
                ps = pspool.tile([P, BL], FP32, tag="ps", name=f"psr1_{m}")
                ps_r1.append(ps)
                for k in range(K2):
                    nc.tensor.matmul(
                        ps[:],
                        wr1_sb[:, k, m * P : (m + 1) * P],
                        pooled_bf[:, k, :],
                        start=(k == 0),
                        stop=False,
                    )
            for m in range(K2):
                ps = ps_r1[m]
                for k in range(K2, KR1):
                    nc.tensor.matmul(
                        ps[:],
                        wr1_sb[:, k, m * P : (m + 1) * P],
                        pooled_bf[:, k, :],
                        start=False,
                        stop=(k == KR1 - 1),
                    )
                # alternate evacuations between ScalarE and VectorE so the
                # short rho epilogue isn't serialized on one engine; DVE does
                # max(x + bias, 0) in a single tensor_scalar op.
                if m % 2 == 0:
                    nc.scalar.activation(
                        r1_sb[:, m, :], ps[:], RELU,
                        bias=br1_sb[:, m : m + 1], scale=1.0,
                    )
                else:
                    nc.vector.tensor_scalar(
                        r1_sb[:, m, :], ps[:],
                        br1_sb[:, m : m + 1], 0.0,
                        mybir.AluOpType.add, mybir.AluOpType.max,
                    )
            for m in range(K2):
                ps = pspool.tile([P, BL], FP32, tag="ps", name=f"psr2_{m}")
                for k in range(K2):
                    nc.tensor.matmul(
                        ps[:],
                        wr2_sb[:, k, m * P : (m + 1) * P],
                        r1_sb[:, k, :],
                        start=(k == 0),
                        stop=(k == K2 - 1),
                    )
                if m % 2 == 0:
                    nc.scalar.activation(
                        out_sb[:, m, :], ps[:], RELU,
                        bias=br2_sb[:, m : m + 1], scale=1.0,
                    )
                else:
                    nc.vector.tensor_scalar(
                        out_sb[:, m, :], ps[:],
                        br2_sb[:, m : m + 1], 0.0,
                        mybir.AluOpType.add, mybir.AluOpType.max,
                    )
                if m == K2 // 2 - 1:
                    # first half of the output leaves while rho2 finishes
                    nc.sync.dma_start(out_d[:, : K2 // 2], out_sb[:, : K2 // 2])
            nc.sync.dma_start(out_d[:, K2 // 2 :], out_sb[:, K2 // 2 :])

    return nc


_CACHE: dict = {}


def get_compiled() -> bacc.Bacc:
    if "nc" not in _CACHE:
        nc = build_program()
        nc.compile()
        _CACHE["nc"] = nc
    return _CACHE["nc"]


def stage_inputs(x, W_phi1, b_phi1, W_phi2, b_phi2, W_rho1, b_rho1, W_rho2, b_rho2):
    """Host-side staging: transpose x, cast to bf16, reshape biases."""

    def wtile(a):
        # [KO*P, H] -> [P, KO, H] with w[p, ko, h] = W[ko*P + p, h]
        a = np.asarray(a, np.float32).astype(NP_BF16)
        ko = a.shape[0] // P
        return np.ascontiguousarray(a.reshape(ko, P, -1).transpose(1, 0, 2))

    def bias(a):
        # [n_tiles*P] -> [P, n_tiles] with b_sb[p, m] = b[m*P + p]
        return np.ascontiguousarray(np.asarray(a, np.float32).reshape(-1, P).T)

    # x[b, n, d] -> xt[b, p, kk, j, n] = x[b, n, kk*256 + j*128 + p]  (fp8)
    xt = np.asarray(x, np.float32).astype(NP_FP8)
    xt = np.ascontiguousarray(xt.reshape(B, N, KK1, 2, P).transpose(0, 4, 2, 3, 1))
    # W1[d, h] -> w1[p, kk, j, h] = W1[kk*256 + j*128 + p, h]  (fp8)
    w1 = np.asarray(W_phi1, np.float32).astype(NP_FP8)
    w1 = np.ascontiguousarray(w1.reshape(KK1, 2, P, D_H).transpose(2, 0, 1, 3))
    shared = {
        "w1": w1,
        "w2": wtile(W_phi2),
        "wr1": wtile(W_rho1),
        "wr2": wtile(W_rho2),
        "b1": bias(b_phi1),
        "b2": bias(b_phi2),
        "br1": bias(b_rho1),
        "br2": bias(b_rho2),
    }
    in_maps = []
    for c in range(N_CORES):
        m = dict(shared)
        m["xt"] = np.ascontiguousarray(xt[c * BL : (c + 1) * BL])
        in_maps.append(m)
    return in_maps


def gather_output(results) -> np.ndarray:
    # per-core out: [P, K2, BL] with out[p, m, s] = r2[m*128+p, s]
    parts = []
    for c in range(N_CORES):
        o = np.asarray(results[c]["out"], np.float32)  # [P, K2, BL]
        parts.append(o.transpose(2, 1, 0).reshape(BL, D_H))  # [BL, D_H]
    return np.concatenate(parts, axis=0)


def run(trace: bool = False, **inputs):
    nc = get_compiled()
    in_maps = stage_inputs(**inputs)
    res = run_bass_kernel_spmd(nc, in_maps, core_ids=list(range(N_CORES)), trace=trace)
    return gather_output(res.results), res


def kernel(**inputs) -> np.ndarray:
    out, _ = run(trace=False, **inputs)
    return out



# revision 9
# speedup vs baseline: 1.1856x; 1.1856x over previous
"""DeepSet encoder (phi MLP -> sum/max pool -> rho MLP) as a Trainium2 Bass kernel.

Sharding: data-parallel over the batch dim. 64 samples -> 8 cores x 8 samples.
Weights are replicated on every core; no cross-core communication.

On-chip layout is feature-major: activations live as [feature_partition,
set_free] tiles so matmul contraction is on the partition dim, the bias is a
per-partition scalar, and pooling over the set dim is a free-axis reduction.

Both phi matmuls run in fp8e4m3 with DoubleRow (2 fp8 rows per PE cell ->
256-row contraction per pass, 2x bf16 throughput). h1 is written directly in
fp8 by the phi1 epilogues. The max pool commutes with relu+bias (both
monotonic), so VectorE reduces the raw PSUM via one fused tensor_tensor_reduce
per tile; the sum pool rides ScalarE's activation accumulator. rho stays fp16.

Self-contained: only relies on the system-installed concourse/bass stack.
"""

import sys

import numpy as np

for _p in ("/opt/trn_rl_repo",):
    if _p not in sys.path:
        sys.path.insert(0, _p)

import ml_dtypes  # noqa: E402

import concourse.bass as bass  # noqa: E402,F401
import concourse.mybir as mybir  # noqa: E402
import concourse.tile as tile  # noqa: E402
from concourse import bacc  # noqa: E402
from concourse.bass_utils import run_bass_kernel_spmd  # noqa: E402

FP16 = mybir.dt.float16
FP32 = mybir.dt.float32
NP_FP16 = np.float16
# fp8e4m3: x ~ N(0,1), W1 ~ U(+-0.044), W2 ~ U(+-0.031), h1 in [0, ~4] all sit
# inside TRN e4m3's +-240 range (subnormals are honored, so W2's sub-2^-6 half
# keeps its 2^-9 quantization step).
FP8 = mybir.dt.float8e4
NP_FP8 = ml_dtypes.float8_e4m3
DOUBLE_ROW = mybir.MatmulPerfMode.DoubleRow

B, N, D_IN, D_H = 64, 512, 512, 1024
N_CORES = 8
BL = B // N_CORES  # samples per core
NPAIR = BL // 2  # xt DMA granularity: sample pairs
P = 128
K2 = D_H // P  # feature tiles of D_H (8)
KK1 = D_IN // 256  # phi1 DoubleRow chunks (2)
KK2 = D_H // 256  # phi2 DoubleRow chunks (4)
KR1 = 2 * D_H // P  # rho1 contraction tiles (16)
N_WARM = 7  # PE warm-up matmuls (cover DMA startup + clock ramp)

RELU = mybir.ActivationFunctionType.Relu
OP_ADD = mybir.AluOpType.add
OP_MAX = mybir.AluOpType.max
NEG_BIG = -3.0e38


def build_program() -> bacc.Bacc:
    nc = bacc.Bacc("TRN2", target_bir_lowering=False, debug=False, num_devices=N_CORES)

    # All staged host-side into the exact SBUF tile layouts so every DMA is
    # contiguous per partition:
    #   xt[g, p, s, kk, j, n] = x[g*2+s, n, kk*256 + j*128 + p]     (fp8)
    #   w1[p, kk, j, h] = W1[kk*256 + j*128 + p, h]                 (fp8)
    #   w2[p, kk, j, h] = W2[kk*256 + j*128 + p, h]                 (fp8)
    #   wr[p, k, h] = Wr1[k*128 + p, h] (k<16) / Wr2[(k-16)*128+p, h]  (fp16)
    #   bias[p, i, m] = b_i[m*128 + p]  (i = b1, b2, br1, br2)      (fp32)
    xt_d = nc.dram_tensor("xt", [NPAIR, P, 2, KK1, 2, N], FP8, kind="ExternalInput").ap()
    w1_d = nc.dram_tensor("w1", [P, KK1, 2, D_H], FP8, kind="ExternalInput").ap()
    w2_d = nc.dram_tensor("w2", [P, KK2, 2, D_H], FP8, kind="ExternalInput").ap()
    wr_d = nc.dram_tensor("wr", [P, KR1 + K2, D_H], FP16, kind="ExternalInput").ap()
    bias_d = nc.dram_tensor("bias", [P, 4, K2], FP32, kind="ExternalInput").ap()
    # out[p, m, s] = r2[m*128 + p, s]  (feature-major, host transposes back)
    out_d = nc.dram_tensor("out", [P, K2, BL], FP32, kind="ExternalOutput").ap()

    with tile.TileContext(nc) as tc:
        with (
            tc.tile_pool(name="const", bufs=1) as cpool,
            tc.tile_pool(name="h1", bufs=2) as h1pool,
            tc.tile_pool(name="h2", bufs=2) as h2pool,
            tc.tile_pool(name="jk", bufs=2) as jkpool,
            tc.tile_pool(name="ps", bufs=8, space="PSUM") as pspool,
        ):
            # --- PE warm-up ---
            # The PE clock needs ~3us of sustained activity to reach 2.4GHz.
            # Burn the DMA-startup window on dummy matmuls over a zeroed tile;
            # phi1(0) then rides the tail of the ramp.
            warm_sb = cpool.tile([P, N], FP16)
            nc.gpsimd.memset(warm_sb[:], 0.0)
            for i in range(N_WARM):
                wps = pspool.tile([P, N], FP32, tag="ps", name=f"warm{i}")
                nc.tensor.matmul(wps[:], warm_sb[:, 0:P], warm_sb[:], start=True, stop=True)

            # --- persistent SBUF state ---
            # The sync sequencer issues one DIRECT2D per ~0.6us, so issue
            # order = time order. First-needed bytes first: w1/xt0 kk-chunks
            # interleaved so phi1(0)'s kk=0 passes can start after ~512KB.
            w1_sb = cpool.tile([P, KK1, 2, D_H], FP8)
            xt_sb = [cpool.tile([P, 2, KK1, 2, N], FP8, name=f"xt{g}") for g in range(NPAIR)]
            for kk in range(KK1):
                nc.sync.dma_start(w1_sb[:, kk], w1_d[:, kk])
                nc.sync.dma_start(xt_sb[0][:, :, kk], xt_d[0, :, :, kk])
            bias_sb = cpool.tile([P, 4, K2], FP32)
            nc.sync.dma_start(bias_sb[:], bias_d)
            w2_sb = cpool.tile([P, KK2, 2, D_H], FP8)
            nc.sync.dma_start(w2_sb[:], w2_d)
            for g in range(1, NPAIR):
                nc.sync.dma_start(xt_sb[g][:], xt_d[g])
            wr_sb = cpool.tile([P, KR1 + K2, D_H], FP16)
            nc.sync.dma_start(wr_sb[:], wr_d)

            pooled = cpool.tile([P, KR1, BL], FP32)  # [0:K2]=sum, [K2:]=raw max
            pooled_bf = cpool.tile([P, KR1, BL], FP16)
            r1_sb = cpool.tile([P, K2, BL], FP16)
            out_sb = cpool.tile([P, K2, BL], FP32)

            def phi1(b):
                # h1 in fp8 DoubleRow pairing for phi2: [P, kk2, j, N] with
                # feature f = kk2*256 + j*128 + p; the phi1 m-tile (m*128+p)
                # lands at (kk2, j) = (m//2, m%2).
                xt = xt_sb[b // 2][:, b % 2]
                h1_sb = h1pool.tile([P, KK2, 2, N], FP8, tag="h1", name=f"h1_{b}")

                def ep(m, ps):
                    # relu(psum + b1) -> fp8; 2 of 8 on ScalarE, rest VectorE
                    dst = h1_sb[:, m // 2, m % 2, :]
                    if m % 4 == 0:
                        nc.scalar.activation(
                            dst, ps[:], RELU, bias=bias_sb[:, 0, m : m + 1], scale=1.0
                        )
                    else:
                        nc.vector.tensor_scalar(
                            dst, ps[:], bias_sb[:, 0, m : m + 1], 0.0, OP_ADD, OP_MAX
                        )

                if b == 0:
                    # kk-major so the first 8 passes only need the kk=0 DMAs
                    ps1 = []
                    for m in range(K2):
                        ps = pspool.tile([P, N], FP32, tag="ps", name=f"ps1_0_{m}")
                        ps1.append(ps)
                        nc.tensor.matmul(
                            ps[:], w1_sb[:, 0, :, m * P : (m + 1) * P], xt[:, 0],
                            perf_mode=DOUBLE_ROW, start=True, stop=False,
                        )
                    for m in range(K2):
                        ps = ps1[m]
                        nc.tensor.matmul(
                            ps[:], w1_sb[:, 1, :, m * P : (m + 1) * P], xt[:, 1],
                            perf_mode=DOUBLE_ROW, start=False, stop=True,
                        )
                        ep(m, ps)
                    return h1_sb
                for m in range(K2):
                    ps = pspool.tile([P, N], FP32, tag="ps", name=f"ps1_{b}_{m}")
                    for kk in range(KK1):
                        nc.tensor.matmul(
                            ps[:], w1_sb[:, kk, :, m * P : (m + 1) * P], xt[:, kk],
                            perf_mode=DOUBLE_ROW, start=(kk == 0), stop=(kk == KK1 - 1),
                        )
                    ep(m, ps)
                return h1_sb

            def phi2(b, h1_sb):
                for m in range(K2):
                    ps = pspool.tile([P, N], FP32, tag="ps", name=f"ps2_{b}_{m}")
                    for kk in range(KK2):
                        nc.tensor.matmul(
                            ps[:], w2_sb[:, kk, :, m * P : (m + 1) * P], h1_sb[:, kk],
                            perf_mode=DOUBLE_ROW, start=(kk == 0), stop=(kk == KK2 - 1),
                        )
                    # sum pool: relu(psum + b2) through ScalarE with the
                    # activation accumulator; the written h2 tile is scratch.
                    h2_sb = h2pool.tile([P, N], FP16, tag="h2", name=f"h2_{b}_{m}")
                    nc.scalar.activation(
                        h2_sb[:], ps[:], RELU,
                        bias=bias_sb[:, 1, m : m + 1], scale=1.0,
                        accum_out=pooled[:, m, b : b + 1],
                    )
                    # max pool over the relu'd h2 on VectorE.
                    nc.vector.tensor_reduce(
                        pooled[:, K2 + m, b : b + 1], h2_sb[:],
                        axis=mybir.AxisListType.X, op=OP_MAX,
                    )
                    if b == BL - 1:
                        # final sample: finish the pooled vectors per tile so
                        # rho1's matmul bursts can chase the epilogue chain.
                        nc.vector.tensor_copy(pooled_bf[:, m, :], pooled[:, m, :])
                        nc.vector.tensor_copy(
                            pooled_bf[:, K2 + m, :], pooled[:, K2 + m, :]
                        )

            # software pipeline: phi1(b+1) is emitted before phi2(b) so the PE
            # never waits on the phi1->phi2 evacuation inside one sample.
            prev_h1 = None
            for b in range(BL):
                h1_sb = phi1(b)
                if prev_h1 is not None:
                    phi2(b - 1, prev_h1)
                prev_h1 = h1_sb
            phi2(BL - 1, prev_h1)

            # --- rho MLP over the 8 pooled vectors (feature-major, free=8) ---
            # All 16 rho1 m-accumulators share one PSUM bank as [P, 8] slices.
            # k-burst order: max half first (ready early via the TTR path),
            # then sum k=0..6 (chasing the ScalarE epilogue chain), then the
            # two stragglers (k=15 needs TTR(7,7), k=7 needs the last cast).
            # One PSUM bank holds all 8 m-accumulators as [P, 8] slices: the
            # first matmul's start zeroes the whole 2KB zero region, so every
            # other pass accumulates (fresh bytes overwrite pending-zero).
            rho1_ps = pspool.tile([P, N], FP32, tag="ps", name="rho1_ps")
            k_order = list(range(K2, KR1 - 1)) + list(range(0, K2 - 1)) + [KR1 - 1, K2 - 1]
            for ki, k in enumerate(k_order):
                for m in range(K2):
                    nc.tensor.matmul(
                        rho1_ps[:, m * BL : (m + 1) * BL],
                        wr_sb[:, k, m * P : (m + 1) * P],
                        pooled_bf[:, k, :],
                        start=(ki == 0 and m == 0),
                        stop=(ki == KR1 - 1 and m == K2 - 1),
                    )
            for m in range(K2):
                ps = rho1_ps[:, m * BL : (m + 1) * BL]
                if m % 2 == 0:
                    nc.scalar.activation(
                        r1_sb[:, m, :], ps, RELU,
                        bias=bias_sb[:, 2, m : m + 1], scale=1.0,
                    )
                else:
                    nc.vector.tensor_scalar(
                        r1_sb[:, m, :], ps,
                        bias_sb[:, 2, m : m + 1], 0.0, OP_ADD, OP_MAX,
                    )
            rho2_ps = pspool.tile([P, N], FP32, tag="ps", name="rho2_ps")
            for k in range(K2):
                for m in range(K2):
                    nc.tensor.matmul(
                        rho2_ps[:, m * BL : (m + 1) * BL],
                        wr_sb[:, KR1 + k, m * P : (m + 1) * P],
                        r1_sb[:, k, :],
                        start=(k == 0 and m == 0),
                        stop=(k == K2 - 1 and m == K2 - 1),
                    )
            for m in range(K2):
                ps = rho2_ps[:, m * BL : (m + 1) * BL]
                if m % 2 == 0:
                    nc.scalar.activation(
                        out_sb[:, m, :], ps, RELU,
                        bias=bias_sb[:, 3, m : m + 1], scale=1.0,
                    )
                else:
                    nc.vector.tensor_scalar(
                        out_sb[:, m, :], ps,
                        bias_sb[:, 3, m : m + 1], 0.0, OP_ADD, OP_MAX,
                    )
                if m == K2 // 2 - 1:
                    # first half of the output leaves while rho2 finishes
                    nc.sync.dma_start(out_d[:, : K2 // 2], out_sb[:, : K2 // 2])
            nc.sync.dma_start(out_d[:, K2 // 2 :], out_sb[:, K2 // 2 :])

    return nc


_CACHE: dict = {}


def get_compiled() -> bacc.Bacc:
    if "nc" not in _CACHE:
        nc = build_program()
        nc.compile()
        _CACHE["nc"] = nc
    return _CACHE["nc"]


def stage_inputs(x, W_phi1, b_phi1, W_phi2, b_phi2, W_rho1, b_rho1, W_rho2, b_rho2):
    """Host-side staging: transpose x, quantize, pack weights/biases."""

    def q8_feedback(a):
        # fp8e4m3 quantization with per-column error feedback down the
        # contraction dim: keeps each column's cumulative quantization error
        # bounded by half a step, so the error stays orthogonal to the large
        # mean component of the activation sums (the sum-pool path).
        a = np.asarray(a, np.float32)
        q = np.empty_like(a)
        err = np.zeros(a.shape[1], np.float32)
        for k in range(a.shape[0]):
            v = a[k] + err
            q[k] = v.astype(NP_FP8).astype(np.float32)
            err = v - q[k]
        return q.astype(NP_FP8)

    def w8(a, kk):
        # [kk*256, H] -> [P, kk, 2, H] with w[p, c, j, h] = W[c*256 + j*128 + p, h]
        a = q8_feedback(a)
        return np.ascontiguousarray(a.reshape(kk, 2, P, -1).transpose(2, 0, 1, 3))

    def w16(a):
        # [KO*P, H] -> [P, KO, H] with w[p, ko, h] = W[ko*P + p, h]
        a = np.asarray(a, np.float32).astype(NP_FP16)
        ko = a.shape[0] // P
        return a.reshape(ko, P, -1).transpose(1, 0, 2)

    def btile(a):
        # [n_tiles*P] -> [P, n_tiles] with b_sb[p, m] = b[m*P + p]
        return np.asarray(a, np.float32).reshape(-1, P).T

    # x[b, n, d] -> xt[g, p, s, kk, j, n] = x[g*2+s, n, kk*256 + j*128 + p]
    xt = np.asarray(x, np.float32).astype(NP_FP8)
    xt = xt.reshape(B // 2, 2, N, KK1, 2, P).transpose(0, 5, 1, 3, 4, 2)
    xt = np.ascontiguousarray(xt)  # [B//2, P, 2, KK1, 2, N]
    wr = np.ascontiguousarray(
        np.concatenate([w16(W_rho1), w16(W_rho2)], axis=1)
    )
    bias = np.ascontiguousarray(
        np.stack([btile(b_phi1), btile(b_phi2), btile(b_rho1), btile(b_rho2)], axis=1)
    )
    shared = {
        "w1": w8(W_phi1, KK1),
        "w2": w8(W_phi2, KK2),
        "wr": wr,
        "bias": bias,
    }
    in_maps = []
    for c in range(N_CORES):
        m = dict(shared)
        m["xt"] = np.ascontiguousarray(xt[c * NPAIR : (c + 1) * NPAIR])
        in_maps.append(m)
    return in_maps


def gather_output(results) -> np.ndarray:
    # per-core out: [P, K2, BL] with out[p, m, s] = r2[m*128+p, s]
    parts = []
    for c in range(N_CORES):
        o = np.asarray(results[c]["out"], np.float32)  # [P, K2, BL]
        parts.append(o.transpose(2, 1, 0).reshape(BL, D_H))  # [BL, D_H]
    return np.concatenate(parts, axis=0)


def run(trace: bool = False, **inputs):
    nc = get_compiled()
    in_maps = stage_inputs(**inputs)
    res = run_bass_kernel_spmd(nc, in_maps, core_ids=list(range(N_CORES)), trace=trace)
    return gather_output(res.results), res


def kernel(**inputs) -> np.ndarray:
    out, _ = run(trace=False, **inputs)
    return out


# revision 16
# speedup vs baseline: 1.2116x; 1.0219x over previous
"""DeepSet encoder (phi MLP -> sum/max pool -> rho MLP) as a Trainium2 Bass kernel.

Sharding: data-parallel over the batch dim. 64 samples -> 8 cores x 8 samples.
Weights are replicated on every core; no cross-core communication.

On-chip layout is feature-major: activations live as [feature_partition,
set_free] tiles so matmul contraction is on the partition dim, the bias is a
per-partition scalar, and pooling over the set dim is a free-axis reduction.

Both phi matmuls run in fp8e4m3 with DoubleRow (2 fp8 rows per PE cell ->
256-row contraction per pass, 2x bf16 throughput). h1 is written directly in
fp8 by the phi1 epilogues. The max pool commutes with relu+bias (both
monotonic), so VectorE reduces the raw PSUM via one fused tensor_tensor_reduce
per tile; the sum pool rides ScalarE's activation accumulator. rho stays fp16.

Self-contained: only relies on the system-installed concourse/bass stack.
"""

import sys

import numpy as np

for _p in ("/opt/trn_rl_repo",):
    if _p not in sys.path:
        sys.path.insert(0, _p)

import ml_dtypes  # noqa: E402

import concourse.bass as bass  # noqa: E402,F401
import concourse.mybir as mybir  # noqa: E402
import concourse.tile as tile  # noqa: E402
from concourse import bacc  # noqa: E402
from concourse.bass_utils import run_bass_kernel_spmd  # noqa: E402

FP16 = mybir.dt.float16
FP32 = mybir.dt.float32
NP_FP16 = np.float16
# fp8e4m3: x ~ N(0,1), W1 ~ U(+-0.044), W2 ~ U(+-0.031), h1 in [0, ~4] all sit
# inside TRN e4m3's +-240 range (subnormals are honored, so W2's sub-2^-6 half
# keeps its 2^-9 quantization step).
FP8 = mybir.dt.float8e4
NP_FP8 = ml_dtypes.float8_e4m3
DOUBLE_ROW = mybir.MatmulPerfMode.DoubleRow

B, N, D_IN, D_H = 64, 512, 512, 1024
N_CORES = 8
BL = B // N_CORES  # samples per core
NPAIR = BL // 2  # xt DMA granularity: sample pairs
P = 128
K2 = D_H // P  # feature tiles of D_H (8)
KK1 = D_IN // 256  # phi1 DoubleRow chunks (2)
KK2 = D_H // 256  # phi2 DoubleRow chunks (4)
KR1 = 2 * D_H // P  # rho1 contraction tiles (16)
N_WARM = 7  # PE warm-up matmuls (cover DMA startup + clock ramp)

RELU = mybir.ActivationFunctionType.Relu
OP_ADD = mybir.AluOpType.add
OP_MAX = mybir.AluOpType.max
NEG_BIG = -3.0e38


def build_program() -> bacc.Bacc:
    nc = bacc.Bacc("TRN2", target_bir_lowering=False, debug=False, num_devices=N_CORES)

    # All staged host-side into the exact SBUF tile layouts so every DMA is
    # contiguous per partition:
    #   xt[g, p, s, kk, j, n] = x[g*2+s, n, kk*256 + j*128 + p]     (fp8)
    #   w1[p, kk, j, h] = W1[kk*256 + j*128 + p, h]                 (fp8)
    #   w2[p, kk, j, h] = W2[kk*256 + j*128 + p, h]                 (fp8)
    #   wr[p, k, h] = Wr1[k*128 + p, h] (k<16) / Wr2[(k-16)*128+p, h]  (fp16)
    #   bias[p, i, m] = b_i[m*128 + p]  (i = b1, b2, br1, br2)      (fp32)
    xt_d = nc.dram_tensor("xt", [NPAIR, P, 2, KK1, 2, N], FP8, kind="ExternalInput").ap()
    w1_d = nc.dram_tensor("w1", [P, KK1, 2, D_H], FP8, kind="ExternalInput").ap()
    w2_d = nc.dram_tensor("w2", [P, KK2, 2, D_H], FP8, kind="ExternalInput").ap()
    wr_d = nc.dram_tensor("wr", [P, KR1 + K2, D_H], FP16, kind="ExternalInput").ap()
    bias_d = nc.dram_tensor("bias", [P, 4, K2], FP32, kind="ExternalInput").ap()
    # out[p, m, s] = r2[m*128 + p, s]  (feature-major, host transposes back)
    out_d = nc.dram_tensor("out", [P, K2, BL], FP32, kind="ExternalOutput").ap()

    with tile.TileContext(nc) as tc:
        with (
            tc.tile_pool(name="const", bufs=1) as cpool,
            tc.tile_pool(name="h1", bufs=2) as h1pool,
            tc.tile_pool(name="h2", bufs=2) as h2pool,
            tc.tile_pool(name="jk", bufs=4) as jkpool,
            tc.tile_pool(name="ps", bufs=8, space="PSUM") as pspool,
        ):
            # --- PE warm-up ---
            # The PE clock needs ~3us of sustained activity to reach 2.4GHz.
            # Burn the DMA-startup window on dummy matmuls over a zeroed tile;
            # phi1(0) then rides the tail of the ramp.
            warm_sb = cpool.tile([P, N], FP16)
            nc.vector.memset(warm_sb[:], 0.0)
            for i in range(N_WARM):
                wps = pspool.tile([P, N], FP32, tag="ps", name=f"warm{i}")
                nc.tensor.matmul(wps[:], warm_sb[:, 0:P], warm_sb[:], start=True, stop=True)

            # --- persistent SBUF state ---
            # The sync sequencer issues one DIRECT2D per ~0.6us, so issue
            # order = time order. First-needed bytes first: w1/xt0 kk-chunks
            # interleaved so phi1(0)'s kk=0 passes can start after ~512KB.
            w1_sb = cpool.tile([P, KK1, 2, D_H], FP8)
            xt_sb = [cpool.tile([P, 2, KK1, 2, N], FP8, name=f"xt{g}") for g in range(NPAIR)]
            for kk in range(KK1):
                nc.sync.dma_start(w1_sb[:, kk], w1_d[:, kk])
                nc.sync.dma_start(xt_sb[0][:, :, kk], xt_d[0, :, :, kk])
            bias_sb = cpool.tile([P, 4, K2], FP32)
            nc.sync.dma_start(bias_sb[:], bias_d)
            w2_sb = cpool.tile([P, KK2, 2, D_H], FP8)
            nc.sync.dma_start(w2_sb[:], w2_d)
            for g in range(1, NPAIR):
                nc.sync.dma_start(xt_sb[g][:], xt_d[g])
            wr_sb = cpool.tile([P, KR1 + K2, D_H], FP16)
            nc.sync.dma_start(wr_sb[:], wr_d)

            pooled = cpool.tile([P, KR1, BL], FP32)  # [0:K2]=sum, [K2:]=max
            pooled_bf = cpool.tile([P, KR1, BL], FP16)
            # rho destinations split per engine (ScalarE: a, VectorE: b) so
            # the two epilogue chains never write the same tile.
            r1a_sb = cpool.tile([P, K2 // 2, BL], FP16)
            r1b_sb = cpool.tile([P, K2 // 2, BL], FP16)
            outa_sb = cpool.tile([P, K2 // 2, BL], FP32)
            outb_sb = cpool.tile([P, K2 // 2, BL], FP32)

            def phi1(b):
                # h1 in fp8 DoubleRow pairing for phi2: [P, kk2, j, N] with
                # feature f = kk2*256 + j*128 + p; the phi1 m-tile (m*128+p)
                # lands at (kk2, j) = (m//2, m%2).
                xt = xt_sb[b // 2][:, b % 2]
                h1_sb = h1pool.tile([P, KK2, 2, N], FP8, tag="h1", name=f"h1_{b}")

                def ep(m, ps):
                    # relu(psum + b1) -> fp8; 3 of 8 on ScalarE, rest VectorE
                    dst = h1_sb[:, m // 2, m % 2, :]
                    if m % 3 == 0:
                        nc.scalar.activation(
                            dst, ps[:], RELU, bias=bias_sb[:, 0, m : m + 1], scale=1.0
                        )
                    else:
                        nc.vector.tensor_scalar(
                            dst, ps[:], bias_sb[:, 0, m : m + 1], 0.0, OP_ADD, OP_MAX
                        )

                if b == 0:
                    # kk-major so the first 8 passes only need the kk=0 DMAs
                    ps1 = []
                    for m in range(K2):
                        ps = pspool.tile([P, N], FP32, tag="ps", name=f"ps1_0_{m}")
                        ps1.append(ps)
                        nc.tensor.matmul(
                            ps[:], w1_sb[:, 0, :, m * P : (m + 1) * P], xt[:, 0],
                            perf_mode=DOUBLE_ROW, start=True, stop=False,
                        )
                    for m in range(K2):
                        ps = ps1[m]
                        nc.tensor.matmul(
                            ps[:], w1_sb[:, 1, :, m * P : (m + 1) * P], xt[:, 1],
                            perf_mode=DOUBLE_ROW, start=False, stop=True,
                        )
                        ep(m, ps)
                    return h1_sb
                for m in range(K2):
                    ps = pspool.tile([P, N], FP32, tag="ps", name=f"ps1_{b}_{m}")
                    for kk in range(KK1):
                        nc.tensor.matmul(
                            ps[:], w1_sb[:, kk, :, m * P : (m + 1) * P], xt[:, kk],
                            perf_mode=DOUBLE_ROW, start=(kk == 0), stop=(kk == KK1 - 1),
                        )
                    ep(m, ps)
                return h1_sb

            def phi2(b, h1_sb):
                for m in range(K2):
                    ps = pspool.tile([P, N], FP32, tag="ps", name=f"ps2_{b}_{m}")
                    for kk in range(KK2):
                        nc.tensor.matmul(
                            ps[:], w2_sb[:, kk, :, m * P : (m + 1) * P], h1_sb[:, kk],
                            perf_mode=DOUBLE_ROW, start=(kk == 0), stop=(kk == KK2 - 1),
                        )
                    # sum pool: relu(psum + b2) through ScalarE with the
                    # activation accumulator; the written h2 tile is scratch.
                    h2_sb = h2pool.tile([P, N], FP16, tag="h2", name=f"h2_{b}_{m}")
                    nc.scalar.activation(
                        h2_sb[:], ps[:], RELU,
                        bias=bias_sb[:, 1, m : m + 1], scale=1.0,
                        accum_out=pooled[:, m, b : b + 1],
                    )
                    # max pool over the relu'd h2 on VectorE.
                    nc.vector.tensor_reduce(
                        pooled[:, K2 + m, b : b + 1], h2_sb[:],
                        axis=mybir.AxisListType.X, op=OP_MAX,
                    )
                    if b == BL - 1:
                        # final sample: finish the pooled vectors per tile so
                        # rho1's matmul bursts can chase the epilogue chain.
                        nc.vector.tensor_copy(pooled_bf[:, m, :], pooled[:, m, :])
                        nc.vector.tensor_copy(
                            pooled_bf[:, K2 + m, :], pooled[:, K2 + m, :]
                        )

            # software pipeline: phi1(b+1) is emitted before phi2(b) so the PE
            # never waits on the phi1->phi2 evacuation inside one sample.
            prev_h1 = None
            for b in range(BL):
                h1_sb = phi1(b)
                if prev_h1 is not None:
                    phi2(b - 1, prev_h1)
                prev_h1 = h1_sb
            phi2(BL - 1, prev_h1)

            # --- rho MLP over the 8 pooled vectors (feature-major, free=8) ---
            # All 16 rho1 m-accumulators share one PSUM bank as [P, 8] slices.
            # k-burst order: max half first (ready early via the TTR path),
            # then sum k=0..6 (chasing the ScalarE epilogue chain), then the
            # two stragglers (k=15 needs TTR(7,7), k=7 needs the last cast).
            # One PSUM bank holds all 8 m-accumulators as [P, 8] slices: the
            # first matmul's start zeroes the whole 2KB zero region, so every
            # other pass accumulates (fresh bytes overwrite pending-zero).
            rho1_ps = pspool.tile([P, N], FP32, tag="ps", name="rho1_ps")
            k_order = list(range(K2, KR1 - 1)) + list(range(0, K2 - 1)) + [KR1 - 1, K2 - 1]
            for ki, k in enumerate(k_order):
                for m in range(K2):
                    nc.tensor.matmul(
                        rho1_ps[:, m * BL : (m + 1) * BL],
                        wr_sb[:, k, m * P : (m + 1) * P],
                        pooled_bf[:, k, :],
                        start=(ki == 0 and m == 0),
                        stop=(ki == KR1 - 1 and m == K2 - 1),
                    )
            # rho epilogues: ScalarE owns m=0..3, VectorE owns m=4..7, each
            # writing its own destination tile so the two chains share no
            # tiles and run fully in parallel. Emission interleaved so both
            # engines start immediately.
            H = K2 // 2
            for mm in range(H):
                for half, eng_m in ((0, mm), (1, H + mm)):
                    ps = rho1_ps[:, eng_m * BL : (eng_m + 1) * BL]
                    dst = (r1a_sb if half == 0 else r1b_sb)[:, eng_m % H, :]
                    if half == 0:
                        nc.scalar.activation(
                            dst, ps, RELU,
                            bias=bias_sb[:, 2, eng_m : eng_m + 1], scale=1.0,
                        )
                    else:
                        nc.vector.tensor_scalar(
                            dst, ps,
                            bias_sb[:, 2, eng_m : eng_m + 1], 0.0, OP_ADD, OP_MAX,
                        )
            rho2_ps = pspool.tile([P, N], FP32, tag="ps", name="rho2_ps")
            for k in range(K2):
                r1k = r1a_sb[:, k, :] if k < H else r1b_sb[:, k - H, :]
                for m in range(K2):
                    nc.tensor.matmul(
                        rho2_ps[:, m * BL : (m + 1) * BL],
                        wr_sb[:, KR1 + k, m * P : (m + 1) * P],
                        r1k,
                        start=(k == 0 and m == 0),
                        stop=(k == K2 - 1 and m == K2 - 1),
                    )
            for mm in range(H):
                for half, eng_m in ((0, mm), (1, H + mm)):
                    ps = rho2_ps[:, eng_m * BL : (eng_m + 1) * BL]
                    dst = (outa_sb if half == 0 else outb_sb)[:, eng_m % H, :]
                    if half == 0:
                        nc.scalar.activation(
                            dst, ps, RELU,
                            bias=bias_sb[:, 3, eng_m : eng_m + 1], scale=1.0,
                        )
                    else:
                        nc.vector.tensor_scalar(
                            dst, ps,
                            bias_sb[:, 3, eng_m : eng_m + 1], 0.0, OP_ADD, OP_MAX,
                        )
            nc.sync.dma_start(out_d[:, :H], outa_sb[:])
            nc.sync.dma_start(out_d[:, H:], outb_sb[:])

    return nc


_CACHE: dict = {}


def get_compiled() -> bacc.Bacc:
    if "nc" not in _CACHE:
        nc = build_program()
        nc.compile()
        _CACHE["nc"] = nc
    return _CACHE["nc"]


def stage_inputs(x, W_phi1, b_phi1, W_phi2, b_phi2, W_rho1, b_rho1, W_rho2, b_rho2):
    """Host-side staging: transpose x, quantize, pack weights/biases."""

    def q8_feedback(a):
        # fp8e4m3 quantization with per-column error feedback down the
        # contraction dim: keeps each column's cumulative quantization error
        # bounded by half a step, so the error stays orthogonal to the large
        # mean component of the activation sums (the sum-pool path).
        a = np.asarray(a, np.float32)
        q = np.empty_like(a)
        err = np.zeros(a.shape[1], np.float32)
        for k in range(a.shape[0]):
            v = a[k] + err
            q[k] = v.astype(NP_FP8).astype(np.float32)
            err = v - q[k]
        return q.astype(NP_FP8)

    def w8(a, kk):
        # [kk*256, H] -> [P, kk, 2, H] with w[p, c, j, h] = W[c*256 + j*128 + p, h]
        a = q8_feedback(a)
        return np.ascontiguousarray(a.reshape(kk, 2, P, -1).transpose(2, 0, 1, 3))

    def w16(a):
        # [KO*P, H] -> [P, KO, H] with w[p, ko, h] = W[ko*P + p, h]
        a = np.asarray(a, np.float32).astype(NP_FP16)
        ko = a.shape[0] // P
        return a.reshape(ko, P, -1).transpose(1, 0, 2)

    def btile(a):
        # [n_tiles*P] -> [P, n_tiles] with b_sb[p, m] = b[m*P + p]
        return np.asarray(a, np.float32).reshape(-1, P).T

    # x[b, n, d] -> xt[g, p, s, kk, j, n] = x[g*2+s, n, kk*256 + j*128 + p]
    xt = np.asarray(x, np.float32).astype(NP_FP8)
    xt = xt.reshape(B // 2, 2, N, KK1, 2, P).transpose(0, 5, 1, 3, 4, 2)
    xt = np.ascontiguousarray(xt)  # [B//2, P, 2, KK1, 2, N]
    wr = np.ascontiguousarray(
        np.concatenate([w16(W_rho1), w16(W_rho2)], axis=1)
    )
    bias = np.ascontiguousarray(
        np.stack([btile(b_phi1), btile(b_phi2), btile(b_rho1), btile(b_rho2)], axis=1)
    )
    shared = {
        "w1": w8(W_phi1, KK1),
        "w2": w8(W_phi2, KK2),
        "wr": wr,
        "bias": bias,
    }
    in_maps = []
    for c in range(N_CORES):
        m = dict(shared)
        m["xt"] = np.ascontiguousarray(xt[c * NPAIR : (c + 1) * NPAIR])
        in_maps.append(m)
    return in_maps


def gather_output(results) -> np.ndarray:
    # per-core out: [P, K2, BL] with out[p, m, s] = r2[m*128+p, s]
    parts = []
    for c in range(N_CORES):
        o = np.asarray(results[c]["out"], np.float32)  # [P, K2, BL]
        parts.append(o.transpose(2, 1, 0).reshape(BL, D_H))  # [BL, D_H]
    return np.concatenate(parts, axis=0)


def run(trace: bool = False, **inputs):
    nc = get_compiled()
    in_maps = stage_inputs(**inputs)
    res = run_bass_kernel_spmd(nc, in_maps, core_ids=list(range(N_CORES)), trace=trace)
    return gather_output(res.results), res


def kernel(**inputs) -> np.ndarray:
    out, _ = run(trace=False, **inputs)
    return out


# revision 20
# speedup vs baseline: 1.4549x; 1.2008x over previous
"""DeepSet encoder (phi MLP -> sum/max pool -> rho MLP) as a Trainium2 Bass kernel.

Sharding: data-parallel over the batch dim. 64 samples -> 8 cores x 8 samples.
Weights are replicated on every core; no cross-core communication.

On-chip layout is feature-major: activations live as [feature_partition,
set_free] tiles so matmul contraction is on the partition dim, the bias is a
per-partition scalar, and pooling over the set dim is a free-axis reduction.

Both phi matmuls run in fp8e4m3 with DoubleRow (2 fp8 rows per PE cell ->
256-row contraction per pass, 2x bf16 throughput). h1 is written directly in
fp8 by the phi1 epilogues. The max pool commutes with relu+bias (both
monotonic), so VectorE reduces the raw PSUM via one fused tensor_tensor_reduce
per tile; the sum pool rides ScalarE's activation accumulator. rho stays fp16.

Self-contained: only relies on the system-installed concourse/bass stack.
"""

import sys

import numpy as np

for _p in ("/opt/trn_rl_repo",):
    if _p not in sys.path:
        sys.path.insert(0, _p)

import ml_dtypes  # noqa: E402

import concourse.bass as bass  # noqa: E402,F401
import concourse.mybir as mybir  # noqa: E402
import concourse.tile as tile  # noqa: E402
from concourse import bacc  # noqa: E402
from concourse.bass_utils import run_bass_kernel_spmd  # noqa: E402

FP16 = mybir.dt.float16
FP32 = mybir.dt.float32
NP_FP16 = np.float16
# fp8e4m3: x ~ N(0,1), W1 ~ U(+-0.044), W2 ~ U(+-0.031), h1 in [0, ~4] all sit
# inside TRN e4m3's +-240 range (subnormals are honored, so W2's sub-2^-6 half
# keeps its 2^-9 quantization step).
FP8 = mybir.dt.float8e4
NP_FP8 = ml_dtypes.float8_e4m3
DOUBLE_ROW = mybir.MatmulPerfMode.DoubleRow

B, N, D_IN, D_H = 64, 512, 512, 1024
N_CORES = 8
BL = B // N_CORES  # samples per core
NPAIR = BL // 2  # xt DMA granularity: sample pairs
P = 128
K2 = D_H // P  # feature tiles of D_H (8)
KK1 = D_IN // 256  # phi1 DoubleRow chunks (2)
KK2 = D_H // 256  # phi2 DoubleRow chunks (4)
KR1 = 2 * D_H // P  # rho1 contraction tiles (16)
N_WARM = 7  # PE warm-up matmuls (cover DMA startup + clock ramp)

RELU = mybir.ActivationFunctionType.Relu
OP_ADD = mybir.AluOpType.add
OP_MAX = mybir.AluOpType.max
NEG_BIG = -3.0e38


def build_program() -> bacc.Bacc:
    nc = bacc.Bacc("TRN2", target_bir_lowering=False, debug=False, num_devices=N_CORES)

    # All staged host-side into the exact SBUF tile layouts so every DMA is
    # contiguous per partition:
    #   xt[g, p, s, kk, j, n] = x[g*2+s, n, kk*256 + j*128 + p]     (fp8)
    #   w1[p, kk, j, h] = W1[kk*256 + j*128 + p, h]                 (fp8)
    #   w2[p, kk, j, h] = W2[kk*256 + j*128 + p, h]                 (fp8)
    #   wr[p, k, h] = Wr1[k*128 + p, h] (k<16) / Wr2[(k-16)*128+p, h]  (fp16)
    #   bias[p, i, m] = b_i[m*128 + p]  (i = b1, b2, br1, br2)      (fp32)
    xt_d = nc.dram_tensor("xt", [NPAIR, P, 2, KK1, 2, N], FP8, kind="ExternalInput").ap()
    w1_d = nc.dram_tensor("w1", [P, KK1, 2, D_H], FP8, kind="ExternalInput").ap()
    w2_d = nc.dram_tensor("w2", [P, KK2, 2, D_H], FP8, kind="ExternalInput").ap()
    wr_d = nc.dram_tensor("wr", [P, KR1 + K2, D_H], FP16, kind="ExternalInput").ap()
    bias_d = nc.dram_tensor("bias", [P, 4, K2], FP32, kind="ExternalInput").ap()
    # out[p, m, s] = r2[m*128 + p, s]  (feature-major, host transposes back)
    out_d = nc.dram_tensor("out", [P, K2, BL], FP32, kind="ExternalOutput").ap()

    with tile.TileContext(nc) as tc:
        with (
            tc.tile_pool(name="const", bufs=1) as cpool,
            tc.tile_pool(name="h1", bufs=2) as h1pool,
            tc.tile_pool(name="h2", bufs=2) as h2pool,
            tc.tile_pool(name="jk", bufs=4) as jkpool,
            tc.tile_pool(name="ps", bufs=8, space="PSUM") as pspool,
        ):
            # --- PE warm-up ---
            # The PE clock needs ~3us of sustained activity to reach 2.4GHz.
            # Burn the DMA-startup window on dummy matmuls over a zeroed tile;
            # phi1(0) then rides the tail of the ramp.
            warm_sb = cpool.tile([P, N], FP16)
            nc.gpsimd.memset(warm_sb[:], 0.0)
            for i in range(N_WARM):
                wps = pspool.tile([P, N], FP32, tag="ps", name=f"warm{i}")
                nc.tensor.matmul(wps[:], warm_sb[:, 0:P], warm_sb[:], start=True, stop=True)

            # --- persistent SBUF state ---
            # The sync sequencer issues one DIRECT2D per ~0.6us, so issue
            # order = time order. First-needed bytes first: w1/xt0 kk-chunks
            # interleaved so phi1(0)'s kk=0 passes can start after ~512KB.
            w1_sb = cpool.tile([P, KK1, 2, D_H], FP8)
            xt_sb = [cpool.tile([P, 2, KK1, 2, N], FP8, name=f"xt{g}") for g in range(NPAIR)]
            for kk in range(KK1):
                nc.sync.dma_start(w1_sb[:, kk], w1_d[:, kk])
                nc.sync.dma_start(xt_sb[0][:, :, kk], xt_d[0, :, :, kk])
            bias_sb = cpool.tile([P, 4, K2], FP32)
            nc.sync.dma_start(bias_sb[:], bias_d)
            w2_sb = cpool.tile([P, KK2, 2, D_H], FP8)
            nc.sync.dma_start(w2_sb[:], w2_d)
            for g in range(1, NPAIR):
                nc.sync.dma_start(xt_sb[g][:], xt_d[g])
            wr_sb = cpool.tile([P, KR1 + K2, D_H], FP16)
            nc.sync.dma_start(wr_sb[:], wr_d)

            pooled = cpool.tile([P, KR1, BL], FP32)  # [0:K2]=sum, [K2:]=max
            pooled_bf = cpool.tile([P, KR1, BL], FP16)
            # rho destinations split per engine (ScalarE: a, VectorE: b) so
            # the two epilogue chains never write the same tile.
            r1a_sb = cpool.tile([P, K2 // 2, BL], FP16)
            r1b_sb = cpool.tile([P, K2 // 2, BL], FP16)
            outa_sb = cpool.tile([P, K2 // 2, BL], FP32)
            outb_sb = cpool.tile([P, K2 // 2, BL], FP32)

            def phi1(b):
                # h1 in fp8 DoubleRow pairing for phi2: [P, kk2, j, N] with
                # feature f = kk2*256 + j*128 + p; the phi1 m-tile (m*128+p)
                # lands at (kk2, j) = (m//2, m%2).
                xt = xt_sb[b // 2][:, b % 2]
                h1_sb = h1pool.tile([P, KK2, 2, N], FP8, tag="h1", name=f"h1_{b}")

                def ep(m, ps):
                    # relu(psum + b1) -> fp8; 3 of 8 on ScalarE, rest VectorE
                    dst = h1_sb[:, m // 2, m % 2, :]
                    if m % 3 == 0:
                        nc.scalar.activation(
                            dst, ps[:], RELU, bias=bias_sb[:, 0, m : m + 1], scale=1.0
                        )
                    else:
                        nc.vector.tensor_scalar(
                            dst, ps[:], bias_sb[:, 0, m : m + 1], 0.0, OP_ADD, OP_MAX
                        )

                if b == 0:
                    # kk-major so the first 8 passes only need the kk=0 DMAs
                    ps1 = []
                    for m in range(K2):
                        ps = pspool.tile([P, N], FP32, tag="ps", name=f"ps1_0_{m}")
                        ps1.append(ps)
                        nc.tensor.matmul(
                            ps[:], w1_sb[:, 0, :, m * P : (m + 1) * P], xt[:, 0],
                            perf_mode=DOUBLE_ROW, start=True, stop=False,
                        )
                    for m in range(K2):
                        ps = ps1[m]
                        nc.tensor.matmul(
                            ps[:], w1_sb[:, 1, :, m * P : (m + 1) * P], xt[:, 1],
                            perf_mode=DOUBLE_ROW, start=False, stop=True,
                        )
                        ep(m, ps)
                    return h1_sb
                for m in range(K2):
                    ps = pspool.tile([P, N], FP32, tag="ps", name=f"ps1_{b}_{m}")
                    for kk in range(KK1):
                        nc.tensor.matmul(
                            ps[:], w1_sb[:, kk, :, m * P : (m + 1) * P], xt[:, kk],
                            perf_mode=DOUBLE_ROW, start=(kk == 0), stop=(kk == KK1 - 1),
                        )
                    ep(m, ps)
                return h1_sb

            def phi2(b, h1_sb):
                for m in range(K2):
                    ps = pspool.tile([P, N], FP32, tag="ps", name=f"ps2_{b}_{m}")
                    for kk in range(KK2):
                        nc.tensor.matmul(
                            ps[:], w2_sb[:, kk, :, m * P : (m + 1) * P], h1_sb[:, kk],
                            perf_mode=DOUBLE_ROW, start=(kk == 0), stop=(kk == KK2 - 1),
                        )
                    # sum pool: relu(psum + b2) through ScalarE with the
                    # activation accumulator; the written h2 tile is scratch.
                    h2_sb = h2pool.tile([P, N], FP16, tag="h2", name=f"h2_{b}_{m}")
                    nc.scalar.activation(
                        h2_sb[:], ps[:], RELU,
                        bias=bias_sb[:, 1, m : m + 1], scale=1.0,
                        accum_out=pooled[:, m, b : b + 1],
                    )
                    # max pool over the relu'd h2 on VectorE.
                    nc.vector.tensor_reduce(
                        pooled[:, K2 + m, b : b + 1], h2_sb[:],
                        axis=mybir.AxisListType.X, op=OP_MAX,
                    )
                    if b == BL - 1:
                        # final sample: finish the pooled vectors per tile so
                        # rho1's matmul bursts can chase the epilogue chain.
                        nc.vector.tensor_copy(pooled_bf[:, m, :], pooled[:, m, :])
                        nc.vector.tensor_copy(
                            pooled_bf[:, K2 + m, :], pooled[:, K2 + m, :]
                        )

            # software pipeline: phi1(b+1) is emitted before phi2(b) so the PE
            # never waits on the phi1->phi2 evacuation inside one sample.
            prev_h1 = None
            for b in range(BL):
                h1_sb = phi1(b)
                if prev_h1 is not None:
                    phi2(b - 1, prev_h1)
                prev_h1 = h1_sb
            phi2(BL - 1, prev_h1)

            # --- rho MLP over the 8 pooled vectors (feature-major, free=8) ---
            # All 16 rho1 m-accumulators share one PSUM bank as [P, 8] slices.
            # k-burst order: max half first (ready early via the TTR path),
            # then sum k=0..6 (chasing the ScalarE epilogue chain), then the
            # two stragglers (k=15 needs TTR(7,7), k=7 needs the last cast).
            # One PSUM bank holds all 8 m-accumulators as [P, 8] slices: the
            # first matmul's start zeroes the whole 2KB zero region, so every
            # other pass accumulates (fresh bytes overwrite pending-zero).
            # Per-engine PSUM banks (A: ScalarE's m=0..3, B: VectorE's m=4..7)
            # so the two epilogue chains never touch the same PSUM tile —
            # same-tile cross-engine access is serialized by the framework.
            rho1_psa = pspool.tile([P, N], FP32, tag="ps", name="rho1_psa")
            rho1_psb = pspool.tile([P, N], FP32, tag="ps", name="rho1_psb")
            H = K2 // 2
            k_order = list(range(K2, KR1 - 1)) + list(range(0, K2 - 1)) + [KR1 - 1, K2 - 1]
            for ki, k in enumerate(k_order):
                for m in range(K2):
                    ps = rho1_psa if m < H else rho1_psb
                    nc.tensor.matmul(
                        ps[:, (m % H) * BL : (m % H + 1) * BL],
                        wr_sb[:, k, m * P : (m + 1) * P],
                        pooled_bf[:, k, :],
                        start=(ki == 0 and m % H == 0),
                        stop=(ki == KR1 - 1 and m % H == H - 1),
                    )
            # rho epilogues: ScalarE owns m=0..3, VectorE owns m=4..7, each
            # writing its own destination tile so the two chains share no
            # tiles and run fully in parallel. Emission interleaved so both
            # engines start immediately.
            for mm in range(H):
                for half, eng_m in ((0, mm), (1, H + mm)):
                    src = rho1_psa if half == 0 else rho1_psb
                    ps = src[:, (eng_m % H) * BL : (eng_m % H + 1) * BL]
                    dst = (r1a_sb if half == 0 else r1b_sb)[:, eng_m % H, :]
                    if half == 0:
                        nc.scalar.activation(
                            dst, ps, RELU,
                            bias=bias_sb[:, 2, eng_m : eng_m + 1], scale=1.0,
                        )
                    else:
                        nc.vector.tensor_scalar(
                            dst, ps,
                            bias_sb[:, 2, eng_m : eng_m + 1], 0.0, OP_ADD, OP_MAX,
                        )
            rho2_psa = pspool.tile([P, N], FP32, tag="ps", name="rho2_psa")
            rho2_psb = pspool.tile([P, N], FP32, tag="ps", name="rho2_psb")
            for k in range(K2):
                r1k = r1a_sb[:, k, :] if k < H else r1b_sb[:, k - H, :]
                for m in range(K2):
                    ps = rho2_psa if m < H else rho2_psb
                    nc.tensor.matmul(
                        ps[:, (m % H) * BL : (m % H + 1) * BL],
                        wr_sb[:, KR1 + k, m * P : (m + 1) * P],
                        r1k,
                        start=(k == 0 and m % H == 0),
                        stop=(k == K2 - 1 and m % H == H - 1),
                    )
            for mm in range(H):
                for half, eng_m in ((0, mm), (1, H + mm)):
                    src = rho2_psa if half == 0 else rho2_psb
                    ps = src[:, (eng_m % H) * BL : (eng_m % H + 1) * BL]
                    dst = (outa_sb if half == 0 else outb_sb)[:, eng_m % H, :]
                    if half == 0:
                        nc.scalar.activation(
                            dst, ps, RELU,
                            bias=bias_sb[:, 3, eng_m : eng_m + 1], scale=1.0,
                        )
                    else:
                        nc.vector.tensor_scalar(
                            dst, ps,
                            bias_sb[:, 3, eng_m : eng_m + 1], 0.0, OP_ADD, OP_MAX,
                        )
            nc.sync.dma_start(out_d[:, :H], outa_sb[:])
            nc.sync.dma_start(out_d[:, H:], outb_sb[:])

    return nc


_CACHE: dict = {}


def get_compiled() -> bacc.Bacc:
    if "nc" not in _CACHE:
        nc = build_program()
        nc.compile()
        _CACHE["nc"] = nc
    return _CACHE["nc"]


def stage_inputs(x, W_phi1, b_phi1, W_phi2, b_phi2, W_rho1, b_rho1, W_rho2, b_rho2):
    """Host-side staging: transpose x, quantize, pack weights/biases."""

    def q8_feedback(a):
        # fp8e4m3 quantization with per-column error feedback down the
        # contraction dim: keeps each column's cumulative quantization error
        # bounded by half a step, so the error stays orthogonal to the large
        # mean component of the activation sums (the sum-pool path).
        a = np.asarray(a, np.float32)
        q = np.empty_like(a)
        err = np.zeros(a.shape[1], np.float32)
        for k in range(a.shape[0]):
            v = a[k] + err
            q[k] = v.astype(NP_FP8).astype(np.float32)
            err = v - q[k]
        return q.astype(NP_FP8)

    def w8(a, kk):
        # [kk*256, H] -> [P, kk, 2, H] with w[p, c, j, h] = W[c*256 + j*128 + p, h]
        a = q8_feedback(a)
        return np.ascontiguousarray(a.reshape(kk, 2, P, -1).transpose(2, 0, 1, 3))

    def w16(a):
        # [KO*P, H] -> [P, KO, H] with w[p, ko, h] = W[ko*P + p, h]
        a = np.asarray(a, np.float32).astype(NP_FP16)
        ko = a.shape[0] // P
        return a.reshape(ko, P, -1).transpose(1, 0, 2)

    def btile(a):
        # [n_tiles*P] -> [P, n_tiles] with b_sb[p, m] = b[m*P + p]
        return np.asarray(a, np.float32).reshape(-1, P).T

    # x[b, n, d] -> xt[g, p, s, kk, j, n] = x[g*2+s, n, kk*256 + j*128 + p]
    xt = np.asarray(x, np.float32).astype(NP_FP8)
    xt = xt.reshape(B // 2, 2, N, KK1, 2, P).transpose(0, 5, 1, 3, 4, 2)
    xt = np.ascontiguousarray(xt)  # [B//2, P, 2, KK1, 2, N]
    wr = np.ascontiguousarray(
        np.concatenate([w16(W_rho1), w16(W_rho2)], axis=1)
    )
    bias = np.ascontiguousarray(
        np.stack([btile(b_phi1), btile(b_phi2), btile(b_rho1), btile(b_rho2)], axis=1)
    )
    shared = {
        "w1": w8(W_phi1, KK1),
        "w2": w8(W_phi2, KK2),
        "wr": wr,
        "bias": bias,
    }
    in_maps = []
    for c in range(N_CORES):
        m = dict(shared)
        m["xt"] = np.ascontiguousarray(xt[c * NPAIR : (c + 1) * NPAIR])
        in_maps.append(m)
    return in_maps


def gather_output(results) -> np.ndarray:
    # per-core out: [P, K2, BL] with out[p, m, s] = r2[m*128+p, s]
    parts = []
    for c in range(N_CORES):
        o = np.asarray(results[c]["out"], np.float32)  # [P, K2, BL]
        parts.append(o.transpose(2, 1, 0).reshape(BL, D_H))  # [BL, D_H]
    return np.concatenate(parts, axis=0)


def run(trace: bool = False, **inputs):
    nc = get_compiled()
    in_maps = stage_inputs(**inputs)
    res = run_bass_kernel_spmd(nc, in_maps, core_ids=list(range(N_CORES)), trace=trace)
    return gather_output(res.results), res


def kernel(**inputs) -> np.ndarray:
    out, _ = run(trace=False, **inputs)
    return out
